# revision 4
# baseline (speedup 1.0000x reference)
"""LIF neuron scan kernel for Trainium2, sharded over 8 NeuronCores.

Device recurrence per time step (f32, fused 2-op form):
    u = v*0.95 + I_t          # one stt: (v mult 0.95) add I_t, in-place over I tile
    v = u * (u < 1.0)         # one stt: (u is_lt 1.0) ... = hard reset

The fused decay differs from the reference's (v - v/20) by <= ~4e-6
(measured over the whole trajectory with synced resets), so the spike
raster can flip only where u lands within that distance of threshold.
The device therefore also emits two Sign threshold planes at 1-EPS and
1+EPS (u8); where they agree the spike bit is provably exact, where
they disagree (|u-1| < EPS, EPS=1e-3 >> 4e-6) the host recomputes those
rows bit-exactly with the reference formula and patches them (~0.02% of
elements land in the band).

Sharding: batch dim B=131072 split into 8 contiguous blocks of 16384
rows. Per core the block is laid out time-major as [128 partitions, 400
steps, 128 neurons] so each step is one [128,128] SBUF slice and DMA
chunks are per-partition contiguous.

Output: two u8 planes per core (a: u > 1-EPS, b: u > 1+EPS); host
decodes spikes = (a == 1), suspect rows = any(a==1 & b!=1), recomputes
suspects exactly, returns f32 raster.
"""

import os
import numpy as np

import concourse.bacc as bacc
import concourse.mybir as mybir
from concourse.tile import TileContext
from concourse.bass_utils import run_bass_kernel_spmd
from concourse.mybir import AluOpType as Op

B, L = 131072, 400
NCORES = 8
RPC = B // NCORES      # rows (neurons) per core
P = 128                # SBUF partitions
J = RPC // P           # neurons per partition = 128 (one step = [P, J] slice)

# Chunk schedule: small first chunks to fill the pipe fast, small last to
# drain fast. Sums to L.
CHUNKS = [2, 6, 16] + [40] * 9 + [8, 4, 2, 2]
assert sum(CHUNKS) == L

G = int(os.environ.get("BASS_LIF_G", "1"))        # interleaved groups
FORMULA = os.environ.get("BASS_LIF_FORMULA", "fused2")
JG = J // G

DECAY_MUL95 = 0.95
DECAY_MUL05 = 0.05
TH = 1.0
EPS = 1e-3

_nc_cache = None


def _build():
    nc = bacc.Bacc(None, target_bir_lowering=False)
    X = nc.dram_tensor("X", [P, L * J], mybir.dt.float32, kind="ExternalInput")
    A = nc.dram_tensor("A", [P, L * J], mybir.dt.uint8, kind="ExternalOutput")
    Bp = nc.dram_tensor("Bq", [P, L * J], mybir.dt.uint8, kind="ExternalOutput")

    with TileContext(nc) as tc:
        with (
            tc.tile_pool(name="state", bufs=1) as state_pool,
            tc.tile_pool(name="io", bufs=3) as io_pool,
            tc.tile_pool(name="tmp", bufs=4) as tmp_pool,
            tc.tile_pool(name="pl", bufs=2) as pl_pool,
        ):
            blo = state_pool.tile([P, 1], mybir.dt.float32, name="blo")
            bhi = state_pool.tile([P, 1], mybir.dt.float32, name="bhi")
            nc.vector.memset(blo[:], float(EPS - TH))
            nc.vector.memset(bhi[:], float(-EPS - TH))
            vg = []
            for g in range(G):
                v = state_pool.tile([P, JG], mybir.dt.float32, name=f"v{g}")
                nc.vector.memset(v[:], 0.0)
                vg.append(v)
            t0 = 0
            for ch, TC in enumerate(CHUNKS):
                base = t0 * J
                t0 += TC
                xin = io_pool.tile([P, TC * J], mybir.dt.float32, name="xin")
                nc.sync.dma_start(xin[:], X[:, base : base + TC * J])
                for t in range(TC):
                    sls = [
                        slice(t * J + g * JG, t * J + (g + 1) * JG) for g in range(G)
                    ]
                    if FORMULA == "fused2":
                        # u = (v * 0.95) + I_t, in place over xin
                        for g in range(G):
                            nc.vector.scalar_tensor_tensor(
                                xin[:, sls[g]], vg[g][:], DECAY_MUL95, xin[:, sls[g]],
                                Op.mult, Op.add,
                            )
                        # v' = (u < 1.0) * u
                        for g in range(G):
                            nc.vector.scalar_tensor_tensor(
                                vg[g][:], xin[:, sls[g]], TH, xin[:, sls[g]],
                                Op.is_lt, Op.mult,
                            )
                    else:  # exact3: nw=(v*0.05)-v; u=I_t-nw; v'=(u<1)*u
                        nws = [
                            tmp_pool.tile([P, JG], mybir.dt.float32, name=f"nw{g}")
                            for g in range(G)
                        ]
                        for g in range(G):
                            nc.vector.scalar_tensor_tensor(
                                nws[g][:], vg[g][:], DECAY_MUL05, vg[g][:],
                                Op.mult, Op.subtract,
                            )
                        for g in range(G):
                            nc.vector.tensor_tensor(
                                xin[:, sls[g]], xin[:, sls[g]], nws[g][:], Op.subtract
                            )
                        for g in range(G):
                            nc.vector.scalar_tensor_tensor(
                                vg[g][:], xin[:, sls[g]], TH, xin[:, sls[g]],
                                Op.is_lt, Op.mult,
                            )
                # Threshold planes on the otherwise-idle ACT engine:
                # pa = Sign(u - (1-EPS)), pb = Sign(u - (1+EPS)) cast to u8.
                # Host decodes spike = (pa == 1); |u-1| < EPS band ->
                # (pa==1) & (pb!=1) -> exact host recompute.
                pa = pl_pool.tile([P, TC * J], mybir.dt.uint8, name="pa")
                pb = pl_pool.tile([P, TC * J], mybir.dt.uint8, name="pb")
                nc.scalar.activation(
                    pa[:], xin[:], mybir.ActivationFunctionType.Sign,
                    bias=blo[:], scale=1.0,
                )
                nc.scalar.activation(
                    pb[:], xin[:], mybir.ActivationFunctionType.Sign,
                    bias=bhi[:], scale=1.0,
                )
                nc.scalar.dma_start(A[:, base : base + TC * J], pa[:])
                nc.scalar.dma_start(Bp[:, base : base + TC * J], pb[:])
    nc.compile()
    return nc


def _get_nc():
    global _nc_cache
    if _nc_cache is None:
        _nc_cache = _build()
    return _nc_cache


def _shard(I):
    # Per-core host transposes run in parallel (numpy releases the GIL
    # during the strided copies).
    from concurrent.futures import ThreadPoolExecutor

    def one(c):
        Ic = I[c * RPC : (c + 1) * RPC]                    # [RPC, L]
        Xc = Ic.reshape(P, J, L).transpose(0, 2, 1)        # [P, L, J] time-major
        return {"X": np.ascontiguousarray(Xc).reshape(P, L * J)}

    with ThreadPoolExecutor(NCORES) as ex:
        return list(ex.map(one, range(NCORES)))


def _unshard_plane(results, key):
    from concurrent.futures import ThreadPoolExecutor

    out = np.empty((B, L), np.uint8)

    def one(c):
        Sc = results[c][key].reshape(P, L, J).transpose(0, 2, 1)   # [P, J, L]
        out[c * RPC : (c + 1) * RPC] = Sc.reshape(RPC, L)

    with ThreadPoolExecutor(NCORES) as ex:
        list(ex.map(one, range(NCORES)))
    return out


def _decode(I, results):
    f32 = np.float32
    pa = _unshard_plane(results, "A")
    pb = _unshard_plane(results, "Bq")
    spikes = pa == 1                      # u > 1-EPS (non-suspect: == u >= 1)
    suspect = spikes & (pb != 1)          # u in (1-EPS, 1+EPS]: too close to call
    rows = np.nonzero(suspect.any(axis=1))[0]
    out = spikes.astype(f32)
    if rows.size:
        # Bit-exact reference recurrence for the suspect rows only.
        Ir = I[rows]
        v = np.zeros(rows.size, f32)
        for t in range(L):
            u = ((v - v * f32(DECAY_MUL05)) + Ir[:, t]).astype(f32)
            s = u >= f32(TH)
            out[rows, t] = s
            v = np.where(s, f32(0.0), u)
    return out


def kernel(I, _trace=False):
    I = np.ascontiguousarray(np.asarray(I), dtype=np.float32)
    assert I.shape == (B, L), I.shape
    nc = _get_nc()
    br = run_bass_kernel_spmd(nc, _shard(I), core_ids=list(range(NCORES)), trace=_trace)
    out = _decode(I, br.results)
    if _trace:
        return out, br
    return out


# revision 5
# speedup vs baseline: 1.9870x; 1.9870x over previous
"""LIF neuron scan kernel for Trainium2, sharded over 8 NeuronCores.

Device recurrence, ONE custom DVE instruction per time step (f32):
    u_t = I_t + 0.95 * (u_{t-1} * (u_{t-1} < 1))
computed in-place over the input tile (state = previous u slice), via a
registered custom DVE op (4 ALU stages).  The mask-multiply is exact, so
this matches the fused 2-op form: u = round(round(0.95*v)+I).

The fused decay differs from the reference's (v - v/20) by <= ~4e-6
over the whole trajectory (measured with synced resets), so the spike
raster can only flip where u lands within that distance of threshold.
The device therefore emits two Sign threshold planes at 1-EPS and 1+EPS
(u8, ACT engine); where they agree the spike bit is provably exact,
where they disagree (|u-1| < EPS, EPS=1e-4 >> 4e-6) the host recomputes
those rows bit-exactly with the reference formula and patches them
(~1e-5 of elements land in the band).

Sharding: batch dim B=131072 split into 8 contiguous blocks of 16384
rows. Per core the block is laid out time-major as [128 partitions, 400
steps, 128 neurons] so each step is one [128,128] SBUF slice and DMA
chunks are per-partition contiguous.
"""

import os
import numpy as np

import concourse.bacc as bacc
import concourse.mybir as mybir
from concourse.tile import TileContext
from concourse.bass_utils import run_bass_kernel_spmd
from concourse.mybir import AluOpType as Op

B, L = 131072, 400
NCORES = 8
RPC = B // NCORES      # rows (neurons) per core
P = 128                # SBUF partitions
J = RPC // P           # neurons per partition = 128 (one step = [P, J] slice)

# Chunk schedule: small first chunks to fill the pipe fast, small last to
# drain fast. Sums to L.
CHUNKS = [2, 6, 16] + [40] * 9 + [8, 4, 2, 2]
assert sum(CHUNKS) == L

G = int(os.environ.get("BASS_LIF_G", "1"))        # interleaved groups
FORMULA = os.environ.get("BASS_LIF_FORMULA", "custom1")
JG = J // G

DECAY_MUL95 = 0.95
DECAY_MUL05 = 0.05
TH = 1.0
EPS = 1e-4

_nc_cache = None
_lif_op = None


def _register_lif_op():
    """Register the fused LIF-step custom DVE op (idempotent):
    out = in0 + (in1 * (in1 < s1)) * s0
    """
    global _lif_op
    if _lif_op is not None:
        return _lif_op
    import concourse.dve_ops as dve_ops
    from concourse.dve_spec import Spec, Src0, Src1, C0, C1, lower
    from concourse.dve_spec import _has_src1
    from concourse.dve_uop import DveOpSpec

    name = "LIF_STEP_ANT"
    for op in dve_ops.OPS:
        if op.name == name:
            _lif_op = op
            return op
    body = Src0 + (Src1 * (Src1 < C1)) * C0
    spec = Spec(
        body=body,
        reference=lambda in0, in1, s0, s1, imm2: (
            in0 + (in1 * (in1 < s1).astype(np.float32)) * np.float32(s0)
        ).astype(np.float32),
    )
    sha = {}
    for ver in ("v3", "v4"):
        sha[ver] = DveOpSpec(
            name=name, opcode=0x1F, uops=lower(spec, ver=ver),
            rd1_en=_has_src1(spec),
        ).sha(ver)
    op = dve_ops.DveOp(name, spec, subdim=False, uops_sha=sha)
    dve_ops.OPS.append(op)
    dve_ops.CUSTOM_DVE_SPECS[name] = spec
    row = dve_ops._CUSTOM_DVE_ROW_BASE + len(dve_ops.OPS) - 1
    assert row < 0x20, "custom-DVE opcode rows exhausted"
    dve_ops._SUB_OPCODE_FOR_NAME[name] = row
    _lif_op = op
    return op


def _build():
    nc = bacc.Bacc(None, target_bir_lowering=False)
    X = nc.dram_tensor("X", [P, L * J], mybir.dt.float32, kind="ExternalInput")
    A = nc.dram_tensor("A", [P, L * J], mybir.dt.uint8, kind="ExternalOutput")
    Bp = nc.dram_tensor("Bq", [P, L * J], mybir.dt.uint8, kind="ExternalOutput")
    lif = _register_lif_op() if FORMULA == "custom1" else None

    with TileContext(nc) as tc:
        with (
            tc.tile_pool(name="state", bufs=1) as state_pool,
            tc.tile_pool(name="io", bufs=3) as io_pool,
            tc.tile_pool(name="pl", bufs=2) as pl_pool,
        ):
            blo = state_pool.tile([P, 1], mybir.dt.float32, name="blo")
            bhi = state_pool.tile([P, 1], mybir.dt.float32, name="bhi")
            nc.vector.memset(blo[:], float(EPS - TH))
            nc.vector.memset(bhi[:], float(-EPS - TH))
            if FORMULA == "custom1":
                # zero "u_{-1}" tile for the first step
                zf = state_pool.tile([P, J], mybir.dt.float32, name="zf")
                nc.vector.memset(zf[:], 0.0)
                prev_tile, prev_t = zf, 0      # u_{t-1} lives at prev_tile[:, prev_t-slice]
                prev_is_zero = True
            else:
                vg = []
                for g in range(G):
                    v = state_pool.tile([P, JG], mybir.dt.float32, name=f"v{g}")
                    nc.vector.memset(v[:], 0.0)
                    vg.append(v)
            t0 = 0
            for ch, TC in enumerate(CHUNKS):
                base = t0 * J
                t0 += TC
                xin = io_pool.tile([P, TC * J], mybir.dt.float32, name="xin")
                nc.sync.dma_start(xin[:], X[:, base : base + TC * J])
                for t in range(TC):
                    sls = [
                        slice(t * J + g * JG, t * J + (g + 1) * JG) for g in range(G)
                    ]
                    if FORMULA == "custom1":
                        if prev_is_zero:
                            psls = [slice(g * JG, (g + 1) * JG) for g in range(G)]
                        else:
                            psls = [
                                slice(prev_t * J + g * JG, prev_t * J + (g + 1) * JG)
                                for g in range(G)
                            ]
                        for g in range(G):
                            nc.vector._custom_dve(
                                lif,
                                out=xin[:, sls[g]],
                                in0=xin[:, sls[g]],
                                in1=prev_tile[:, psls[g]],
                                s0=DECAY_MUL95,
                                s1=TH,
                            )
                        prev_tile, prev_t, prev_is_zero = xin, t, False
                    else:  # "fused2": u = (v*0.95)+I ; v' = (u<1)*u
                        for g in range(G):
                            nc.vector.scalar_tensor_tensor(
                                xin[:, sls[g]], vg[g][:], DECAY_MUL95, xin[:, sls[g]],
                                Op.mult, Op.add,
                            )
                        for g in range(G):
                            nc.vector.scalar_tensor_tensor(
                                vg[g][:], xin[:, sls[g]], TH, xin[:, sls[g]],
                                Op.is_lt, Op.mult,
                            )
                # Threshold planes on the otherwise-idle ACT engine:
                # pa = Sign(u - (1-EPS)), pb = Sign(u - (1+EPS)) cast to u8.
                pa = pl_pool.tile([P, TC * J], mybir.dt.uint8, name="pa")
                pb = pl_pool.tile([P, TC * J], mybir.dt.uint8, name="pb")
                nc.scalar.activation(
                    pa[:], xin[:], mybir.ActivationFunctionType.Sign,
                    bias=blo[:], scale=1.0,
                )
                nc.scalar.activation(
                    pb[:], xin[:], mybir.ActivationFunctionType.Sign,
                    bias=bhi[:], scale=1.0,
                )
                nc.scalar.dma_start(A[:, base : base + TC * J], pa[:])
                nc.scalar.dma_start(Bp[:, base : base + TC * J], pb[:])
    nc.compile()
    return nc


def _get_nc():
    global _nc_cache
    if _nc_cache is None:
        _nc_cache = _build()
    return _nc_cache


def _shard(I):
    # Per-core host transposes run in parallel (numpy releases the GIL
    # during the strided copies).
    from concurrent.futures import ThreadPoolExecutor

    def one(c):
        Ic = I[c * RPC : (c + 1) * RPC]                    # [RPC, L]
        Xc = Ic.reshape(P, J, L).transpose(0, 2, 1)        # [P, L, J] time-major
        return {"X": np.ascontiguousarray(Xc).reshape(P, L * J)}

    with ThreadPoolExecutor(NCORES) as ex:
        return list(ex.map(one, range(NCORES)))


def _unshard_plane(results, key):
    from concurrent.futures import ThreadPoolExecutor

    out = np.empty((B, L), np.uint8)

    def one(c):
        Sc = results[c][key].reshape(P, L, J).transpose(0, 2, 1)   # [P, J, L]
        out[c * RPC : (c + 1) * RPC] = Sc.reshape(RPC, L)

    with ThreadPoolExecutor(NCORES) as ex:
        list(ex.map(one, range(NCORES)))
    return out


def _decode(I, results):
    f32 = np.float32
    pa = _unshard_plane(results, "A")
    pb = _unshard_plane(results, "Bq")
    spikes = pa == 1                      # u > 1-EPS (non-suspect: == u >= 1)
    suspect = spikes & (pb != 1)          # u in (1-EPS, 1+EPS]: too close to call
    rows = np.nonzero(suspect.any(axis=1))[0]
    out = spikes.astype(f32)
    if rows.size:
        # Bit-exact reference recurrence for the suspect rows only.
        Ir = I[rows]
        v = np.zeros(rows.size, f32)
        for t in range(L):
            u = ((v - v * f32(DECAY_MUL05)) + Ir[:, t]).astype(f32)
            s = u >= f32(TH)
            out[rows, t] = s
            v = np.where(s, f32(0.0), u)
    return out


def kernel(I, _trace=False):
    I = np.ascontiguousarray(np.asarray(I), dtype=np.float32)
    assert I.shape == (B, L), I.shape
    nc = _get_nc()
    br = run_bass_kernel_spmd(nc, _shard(I), core_ids=list(range(NCORES)), trace=_trace)
    out = _decode(I, br.results)
    if _trace:
        return out, br
    return out


# revision 6
# speedup vs baseline: 2.0601x; 1.0368x over previous
"""LIF neuron scan kernel for Trainium2, sharded over 8 NeuronCores.

Device recurrence, ONE custom DVE instruction per time step (f32):
    u_t = I_t + 0.95 * (u_{t-1} * (u_{t-1} < 1))
computed in-place over the input tile (state = previous u slice), via a
registered custom DVE op (4 ALU stages).  The mask-multiply is exact, so
this matches the fused form u = round(round(0.95*v)+I).

The fused decay differs from the reference's (v - v/20) by <= ~4e-6
over the whole trajectory (measured with synced resets), so the spike
raster can only flip where u lands within that distance of threshold.
The device emits one fp8-e4m3 plane sig = sigmoid(8192*(u-1)) (ACT
engine): bytes decode monotonically in u, sigma > 0.5 <=> u > 1, and any
u within 3.8e-6 of threshold maps within 0.008 of sigma=0.5 -- far
inside one fp8 quantum (0.0625) -- so the host flags bytes near 0.5 as
suspects and recomputes those rows bit-exactly with the reference
formula (~1e-5 of rows).

Sharding: batch dim B=131072 split into 8 contiguous blocks of 16384
rows. Per core the block is laid out time-major as [128 partitions, 400
steps, 128 neurons] so each step is one [128,128] SBUF slice and DMA
chunks are per-partition contiguous.
"""

import os
import numpy as np

import concourse.bacc as bacc
import concourse.mybir as mybir
from concourse.tile import TileContext
from concourse.bass_utils import run_bass_kernel_spmd
from concourse.mybir import AluOpType as Op

B, L = 131072, 400
NCORES = 8
RPC = B // NCORES      # rows (neurons) per core
P = 128                # SBUF partitions
J = RPC // P           # neurons per partition = 128 (one step = [P, J] slice)

# Chunk schedule: small first chunks to fill the pipe fast, small last to
# drain fast. Sums to L.
CHUNKS = [2, 6, 16] + [64] * 5 + [40, 12, 4]
assert sum(CHUNKS) == L

G = int(os.environ.get("BASS_LIF_G", "2"))        # interleaved groups
PLANES = os.environ.get("BASS_LIF_PLANES", "sig8")
JG = J // G

DECAY_MUL95 = 0.95
DECAY_MUL05 = 0.05
TH = 1.0
EPS = 1e-4            # sign2 band half-width
KSIG = 8192.0         # sig8 sigmoid sharpness

_nc_cache = None
_lif_op = None


def _register_lif_op():
    """Register the fused LIF-step custom DVE op (idempotent):
    out = in0 + (in1 * (in1 < s1)) * s0
    """
    global _lif_op
    if _lif_op is not None:
        return _lif_op
    import concourse.dve_ops as dve_ops
    from concourse.dve_spec import Spec, Src0, Src1, C0, C1, lower
    from concourse.dve_spec import _has_src1
    from concourse.dve_uop import DveOpSpec

    name = "LIF_STEP_ANT"
    for op in dve_ops.OPS:
        if op.name == name:
            _lif_op = op
            return op
    body = Src0 + (Src1 * (Src1 < C1)) * C0
    spec = Spec(
        body=body,
        reference=lambda in0, in1, s0, s1, imm2: (
            in0 + (in1 * (in1 < s1).astype(np.float32)) * np.float32(s0)
        ).astype(np.float32),
    )
    sha = {}
    for ver in ("v3", "v4"):
        sha[ver] = DveOpSpec(
            name=name, opcode=0x1F, uops=lower(spec, ver=ver),
            rd1_en=_has_src1(spec),
        ).sha(ver)
    op = dve_ops.DveOp(name, spec, subdim=False, uops_sha=sha)
    dve_ops.OPS.append(op)
    dve_ops.CUSTOM_DVE_SPECS[name] = spec
    row = dve_ops._CUSTOM_DVE_ROW_BASE + len(dve_ops.OPS) - 1
    assert row < 0x20, "custom-DVE opcode rows exhausted"
    dve_ops._SUB_OPCODE_FOR_NAME[name] = row
    _lif_op = op
    return op


def _build():
    nc = bacc.Bacc(None, target_bir_lowering=False)
    X = nc.dram_tensor("X", [P, L * J], mybir.dt.float32, kind="ExternalInput")
    if PLANES == "sig8":
        S8 = nc.dram_tensor("S8", [P, L * J], mybir.dt.float8e4, kind="ExternalOutput")
    else:
        A = nc.dram_tensor("A", [P, L * J], mybir.dt.uint8, kind="ExternalOutput")
        Bp = nc.dram_tensor("Bq", [P, L * J], mybir.dt.uint8, kind="ExternalOutput")
    lif = _register_lif_op()

    with TileContext(nc) as tc:
        with (
            tc.tile_pool(name="state", bufs=1) as state_pool,
            tc.tile_pool(name="io", bufs=3) as io_pool,
            tc.tile_pool(name="pl", bufs=2) as pl_pool,
        ):
            if PLANES == "sig8":
                bsg = state_pool.tile([P, 1], mybir.dt.float32, name="bsg")
                nc.vector.memset(bsg[:], -KSIG)
            else:
                blo = state_pool.tile([P, 1], mybir.dt.float32, name="blo")
                bhi = state_pool.tile([P, 1], mybir.dt.float32, name="bhi")
                nc.vector.memset(blo[:], float(EPS - TH))
                nc.vector.memset(bhi[:], float(-EPS - TH))
            # zero "u_{-1}" tile for the first step
            zf = state_pool.tile([P, J], mybir.dt.float32, name="zf")
            nc.vector.memset(zf[:], 0.0)
            prev_tile, prev_t = zf, 0
            prev_is_zero = True
            t0 = 0
            for ch, TC in enumerate(CHUNKS):
                base = t0 * J
                t0 += TC
                xin = io_pool.tile([P, TC * J], mybir.dt.float32, name="xin")
                nc.sync.dma_start(xin[:], X[:, base : base + TC * J])
                for t in range(TC):
                    sls = [
                        slice(t * J + g * JG, t * J + (g + 1) * JG) for g in range(G)
                    ]
                    if prev_is_zero:
                        psls = [slice(g * JG, (g + 1) * JG) for g in range(G)]
                    else:
                        psls = [
                            slice(prev_t * J + g * JG, prev_t * J + (g + 1) * JG)
                            for g in range(G)
                        ]
                    for g in range(G):
                        nc.vector._custom_dve(
                            lif,
                            out=xin[:, sls[g]],
                            in0=xin[:, sls[g]],
                            in1=prev_tile[:, psls[g]],
                            s0=DECAY_MUL95,
                            s1=TH,
                        )
                    prev_tile, prev_t, prev_is_zero = xin, t, False
                if PLANES == "sig8":
                    # sig = sigmoid(KSIG*(u-1)) -> fp8: monotone byte code of u;
                    # bytes near 0.5 (|u-1| <~ 4e-5) are the host-repair band.
                    ps = pl_pool.tile([P, TC * J], mybir.dt.float8e4, name="ps")
                    nc.scalar.activation(
                        ps[:], xin[:], mybir.ActivationFunctionType.Sigmoid,
                        bias=bsg[:], scale=KSIG,
                    )
                    nc.scalar.dma_start(S8[:, base : base + TC * J], ps[:])
                else:
                    pa = pl_pool.tile([P, TC * J], mybir.dt.uint8, name="pa")
                    pb = pl_pool.tile([P, TC * J], mybir.dt.uint8, name="pb")
                    nc.scalar.activation(
                        pa[:], xin[:], mybir.ActivationFunctionType.Sign,
                        bias=blo[:], scale=1.0,
                    )
                    nc.scalar.activation(
                        pb[:], xin[:], mybir.ActivationFunctionType.Sign,
                        bias=bhi[:], scale=1.0,
                    )
                    nc.scalar.dma_start(A[:, base : base + TC * J], pa[:])
                    nc.scalar.dma_start(Bp[:, base : base + TC * J], pb[:])
    nc.compile()
    return nc


def _get_nc():
    global _nc_cache
    if _nc_cache is None:
        _nc_cache = _build()
    return _nc_cache


def _shard(I):
    # Per-core host transposes run in parallel (numpy releases the GIL
    # during the strided copies).
    from concurrent.futures import ThreadPoolExecutor

    def one(c):
        Ic = I[c * RPC : (c + 1) * RPC]                    # [RPC, L]
        Xc = Ic.reshape(P, J, L).transpose(0, 2, 1)        # [P, L, J] time-major
        return {"X": np.ascontiguousarray(Xc).reshape(P, L * J)}

    with ThreadPoolExecutor(NCORES) as ex:
        return list(ex.map(one, range(NCORES)))


def _unshard_plane(results, key):
    from concurrent.futures import ThreadPoolExecutor

    out = np.empty((B, L), np.uint8)

    def one(c):
        r = np.asarray(results[c][key])
        if r.dtype != np.uint8:
            r = r.view(np.uint8) if r.dtype.itemsize == 1 else r.astype(np.uint8)
        Sc = r.reshape(P, L, J).transpose(0, 2, 1)         # [P, J, L]
        out[c * RPC : (c + 1) * RPC] = Sc.reshape(RPC, L)

    with ThreadPoolExecutor(NCORES) as ex:
        list(ex.map(one, range(NCORES)))
    return out


def _f8e4m3_lut():
    """byte -> float32 value of fp8 e4m3 (bias 7)."""
    b = np.arange(256, dtype=np.uint32)
    sign = np.where(b >> 7, -1.0, 1.0)
    e = (b >> 3) & 0xF
    m = b & 0x7
    val = np.where(
        e == 0,
        (m / 8.0) * 2.0 ** (-6),
        (1.0 + m / 8.0) * (2.0 ** (e.astype(np.int32) - 7)),
    )
    return (sign * val).astype(np.float32)


def _decode(I, results):
    f32 = np.float32
    if PLANES == "sig8":
        raw = _unshard_plane(results, "S8")
        val = _f8e4m3_lut()[raw]
        spikes = val > f32(0.5)
        suspect = np.abs(val - f32(0.5)) <= f32(0.05)
    else:
        pa = _unshard_plane(results, "A")
        pb = _unshard_plane(results, "Bq")
        spikes = pa == 1
        suspect = spikes & (pb != 1)
    rows = np.nonzero(suspect.any(axis=1))[0]
    out = spikes.astype(f32)
    if rows.size:
        # Bit-exact reference recurrence for the suspect rows only.
        Ir = I[rows]
        v = np.zeros(rows.size, f32)
        for t in range(L):
            u = ((v - v * f32(DECAY_MUL05)) + Ir[:, t]).astype(f32)
            s = u >= f32(TH)
            out[rows, t] = s
            v = np.where(s, f32(0.0), u)
    return out


def kernel(I, _trace=False):
    I = np.ascontiguousarray(np.asarray(I), dtype=np.float32)
    assert I.shape == (B, L), I.shape
    nc = _get_nc()
    br = run_bass_kernel_spmd(nc, _shard(I), core_ids=list(range(NCORES)), trace=_trace)
    out = _decode(I, br.results)
    if _trace:
        return out, br
    return out


# revision 8
# speedup vs baseline: 2.0913x; 1.0151x over previous
"""LIF neuron scan kernel for Trainium2, sharded over 8 NeuronCores.

Device recurrence, ONE custom DVE instruction per time step (f32):
    u_t = I_t + 0.95 * (u_{t-1} * (u_{t-1} < 1))
computed in-place over the input tile (state = previous u slice), via a
registered custom DVE op (4 ALU stages).  The mask-multiply is exact, so
this matches the fused form u = round(round(0.95*v)+I).

The fused decay differs from the reference's (v - v/20) by <= ~4e-6
over the whole trajectory (measured with synced resets), so the spike
raster can only flip where u lands within that distance of threshold.
The device emits one fp8-e4m3 plane sig = sigmoid(8192*(u-1)) (ACT
engine): bytes decode monotonically in u, sigma > 0.5 <=> u > 1, and any
u within 3.8e-6 of threshold maps within 0.008 of sigma=0.5 -- far
inside one fp8 quantum (0.0625) -- so the host flags bytes near 0.5 as
suspects and recomputes those rows bit-exactly with the reference
formula (~1e-5 of rows).

Sharding: batch dim B=131072 split into 8 contiguous blocks of 16384
rows. Per core the block is laid out time-major as [128 partitions, 400
steps, 128 neurons] so each step is one [128,128] SBUF slice and DMA
chunks are per-partition contiguous.
"""

import os
import numpy as np

import concourse.bacc as bacc
import concourse.mybir as mybir
from concourse.tile import TileContext
from concourse.bass_utils import run_bass_kernel_spmd
from concourse.mybir import AluOpType as Op

B, L = 131072, 400
NCORES = 8
RPC = B // NCORES      # rows (neurons) per core
P = 128                # SBUF partitions
J = RPC // P           # neurons per partition = 128 (one step = [P, J] slice)

# Chunk schedule: small first chunks to fill the pipe fast, small last to
# drain fast. Sums to L.
CHUNKS = [2, 6, 16, 32] + [64] * 5 + [16, 8]
assert sum(CHUNKS) == L

G = int(os.environ.get("BASS_LIF_G", "2"))        # interleaved groups
PLANES = os.environ.get("BASS_LIF_PLANES", "sig8")
JG = J // G

DECAY_MUL95 = 0.95
DECAY_MUL05 = 0.05
TH = 1.0
EPS = 1e-4            # sign2 band half-width
KSIG = 8192.0         # sig8 sigmoid sharpness

_nc_cache = None
_lif_op = None


def _register_lif_op():
    """Register the fused LIF-step custom DVE op (idempotent):
    out = in0 + (in1 * (in1 < s1)) * s0
    """
    global _lif_op
    if _lif_op is not None:
        return _lif_op
    import concourse.dve_ops as dve_ops
    from concourse.dve_spec import Spec, Src0, Src1, C0, C1, lower
    from concourse.dve_spec import _has_src1
    from concourse.dve_uop import DveOpSpec

    name = "LIF_STEP_ANT"
    for op in dve_ops.OPS:
        if op.name == name:
            _lif_op = op
            return op
    body = Src0 + (Src1 * (Src1 < C1)) * C0
    spec = Spec(
        body=body,
        reference=lambda in0, in1, s0, s1, imm2: (
            in0 + (in1 * (in1 < s1).astype(np.float32)) * np.float32(s0)
        ).astype(np.float32),
    )
    sha = {}
    for ver in ("v3", "v4"):
        sha[ver] = DveOpSpec(
            name=name, opcode=0x1F, uops=lower(spec, ver=ver),
            rd1_en=_has_src1(spec),
        ).sha(ver)
    op = dve_ops.DveOp(name, spec, subdim=False, uops_sha=sha)
    dve_ops.OPS.append(op)
    dve_ops.CUSTOM_DVE_SPECS[name] = spec
    row = dve_ops._CUSTOM_DVE_ROW_BASE + len(dve_ops.OPS) - 1
    assert row < 0x20, "custom-DVE opcode rows exhausted"
    dve_ops._SUB_OPCODE_FOR_NAME[name] = row
    _lif_op = op
    return op


def _build():
    nc = bacc.Bacc(None, target_bir_lowering=False)
    X = nc.dram_tensor("X", [P, L * J], mybir.dt.float32, kind="ExternalInput")
    if PLANES == "sig8":
        S8 = nc.dram_tensor("S8", [P, L * J], mybir.dt.float8e4, kind="ExternalOutput")
    else:
        A = nc.dram_tensor("A", [P, L * J], mybir.dt.uint8, kind="ExternalOutput")
        Bp = nc.dram_tensor("Bq", [P, L * J], mybir.dt.uint8, kind="ExternalOutput")
    lif = _register_lif_op()

    with TileContext(nc) as tc:
        with (
            tc.tile_pool(name="state", bufs=1) as state_pool,
            tc.tile_pool(name="io", bufs=4) as io_pool,
            tc.tile_pool(name="pl", bufs=2) as pl_pool,
        ):
            if PLANES == "sig8":
                bsg = state_pool.tile([P, 1], mybir.dt.float32, name="bsg")
                nc.vector.memset(bsg[:], -KSIG)
            else:
                blo = state_pool.tile([P, 1], mybir.dt.float32, name="blo")
                bhi = state_pool.tile([P, 1], mybir.dt.float32, name="bhi")
                nc.vector.memset(blo[:], float(EPS - TH))
                nc.vector.memset(bhi[:], float(-EPS - TH))
            # zero "u_{-1}" tile for the first step
            zf = state_pool.tile([P, J], mybir.dt.float32, name="zf")
            nc.vector.memset(zf[:], 0.0)
            prev_tile, prev_t = zf, 0
            prev_is_zero = True
            t0 = 0
            for ch, TC in enumerate(CHUNKS):
                base = t0 * J
                t0 += TC
                xin = io_pool.tile([P, TC * J], mybir.dt.float32, name="xin")
                nc.sync.dma_start(xin[:], X[:, base : base + TC * J])
                for t in range(TC):
                    sls = [
                        slice(t * J + g * JG, t * J + (g + 1) * JG) for g in range(G)
                    ]
                    if prev_is_zero:
                        psls = [slice(g * JG, (g + 1) * JG) for g in range(G)]
                    else:
                        psls = [
                            slice(prev_t * J + g * JG, prev_t * J + (g + 1) * JG)
                            for g in range(G)
                        ]
                    for g in range(G):
                        nc.vector._custom_dve(
                            lif,
                            out=xin[:, sls[g]],
                            in0=xin[:, sls[g]],
                            in1=prev_tile[:, psls[g]],
                            s0=DECAY_MUL95,
                            s1=TH,
                        )
                    prev_tile, prev_t, prev_is_zero = xin, t, False
                if PLANES == "sig8":
                    # sig = sigmoid(KSIG*(u-1)) -> fp8: monotone byte code of u;
                    # bytes near 0.5 (|u-1| <~ 4e-5) are the host-repair band.
                    ps = pl_pool.tile([P, TC * J], mybir.dt.float8e4, name="ps")
                    nc.scalar.activation(
                        ps[:], xin[:], mybir.ActivationFunctionType.Sigmoid,
                        bias=bsg[:], scale=KSIG,
                    )
                    nc.scalar.dma_start(S8[:, base : base + TC * J], ps[:])
                else:
                    pa = pl_pool.tile([P, TC * J], mybir.dt.uint8, name="pa")
                    pb = pl_pool.tile([P, TC * J], mybir.dt.uint8, name="pb")
                    nc.scalar.activation(
                        pa[:], xin[:], mybir.ActivationFunctionType.Sign,
                        bias=blo[:], scale=1.0,
                    )
                    nc.scalar.activation(
                        pb[:], xin[:], mybir.ActivationFunctionType.Sign,
                        bias=bhi[:], scale=1.0,
                    )
                    nc.scalar.dma_start(A[:, base : base + TC * J], pa[:])
                    nc.scalar.dma_start(Bp[:, base : base + TC * J], pb[:])
    nc.compile()
    return nc


def _get_nc():
    global _nc_cache
    if _nc_cache is None:
        _nc_cache = _build()
    return _nc_cache


def _shard(I):
    # Per-core host transposes run in parallel (numpy releases the GIL
    # during the strided copies).
    from concurrent.futures import ThreadPoolExecutor

    def one(c):
        Ic = I[c * RPC : (c + 1) * RPC]                    # [RPC, L]
        Xc = Ic.reshape(P, J, L).transpose(0, 2, 1)        # [P, L, J] time-major
        return {"X": np.ascontiguousarray(Xc).reshape(P, L * J)}

    with ThreadPoolExecutor(NCORES) as ex:
        return list(ex.map(one, range(NCORES)))


def _unshard_plane(results, key):
    from concurrent.futures import ThreadPoolExecutor

    out = np.empty((B, L), np.uint8)

    def one(c):
        r = np.asarray(results[c][key])
        if r.dtype != np.uint8:
            r = r.view(np.uint8) if r.dtype.itemsize == 1 else r.astype(np.uint8)
        Sc = r.reshape(P, L, J).transpose(0, 2, 1)         # [P, J, L]
        out[c * RPC : (c + 1) * RPC] = Sc.reshape(RPC, L)

    with ThreadPoolExecutor(NCORES) as ex:
        list(ex.map(one, range(NCORES)))
    return out


def _f8e4m3_lut():
    """byte -> float32 value of fp8 e4m3 (bias 7)."""
    b = np.arange(256, dtype=np.uint32)
    sign = np.where(b >> 7, -1.0, 1.0)
    e = (b >> 3) & 0xF
    m = b & 0x7
    val = np.where(
        e == 0,
        (m / 8.0) * 2.0 ** (-6),
        (1.0 + m / 8.0) * (2.0 ** (e.astype(np.int32) - 7)),
    )
    return (sign * val).astype(np.float32)


def _decode(I, results):
    f32 = np.float32
    if PLANES == "sig8":
        raw = _unshard_plane(results, "S8")
        val = _f8e4m3_lut()[raw]
        spikes = val > f32(0.5)
        suspect = np.abs(val - f32(0.5)) <= f32(0.05)
    else:
        pa = _unshard_plane(results, "A")
        pb = _unshard_plane(results, "Bq")
        spikes = pa == 1
        suspect = spikes & (pb != 1)
    rows = np.nonzero(suspect.any(axis=1))[0]
    out = spikes.astype(f32)
    if rows.size:
        # Bit-exact reference recurrence for the suspect rows only.
        Ir = I[rows]
        v = np.zeros(rows.size, f32)
        for t in range(L):
            u = ((v - v * f32(DECAY_MUL05)) + Ir[:, t]).astype(f32)
            s = u >= f32(TH)
            out[rows, t] = s
            v = np.where(s, f32(0.0), u)
    return out


def kernel(I, _trace=False):
    I = np.ascontiguousarray(np.asarray(I), dtype=np.float32)
    assert I.shape == (B, L), I.shape
    nc = _get_nc()
    br = run_bass_kernel_spmd(nc, _shard(I), core_ids=list(range(NCORES)), trace=_trace)
    out = _decode(I, br.results)
    if _trace:
        return out, br
    return out


# revision 12
# speedup vs baseline: 2.0945x; 1.0015x over previous
"""LIF neuron scan kernel for Trainium2, sharded over 8 NeuronCores.

Device recurrence, ONE custom DVE instruction per time step (f32):
    u_t = I_t + 0.95 * (u_{t-1} * (u_{t-1} < 1))
computed in-place over the input tile (state = previous u slice), via a
registered custom DVE op (4 ALU stages).  The mask-multiply is exact, so
this matches the fused form u = round(round(0.95*v)+I).

The fused decay differs from the reference's (v - v/20) by <= ~4e-6
over the whole trajectory (measured with synced resets), so the spike
raster can only flip where u lands within that distance of threshold.
The device emits one fp8-e4m3 plane sig = sigmoid(8192*(u-1)) (ACT
engine): bytes decode monotonically in u, sigma > 0.5 <=> u > 1, and any
u within 3.8e-6 of threshold maps within 0.008 of sigma=0.5 -- far
inside one fp8 quantum (0.0625) -- so the host flags bytes near 0.5 as
suspects and recomputes those rows bit-exactly with the reference
formula (~1e-5 of rows).

Sharding: batch dim B=131072 split into 8 contiguous blocks of 16384
rows. Per core the block is laid out time-major as [128 partitions, 400
steps, 128 neurons] so each step is one [128,128] SBUF slice and DMA
chunks are per-partition contiguous.
"""

import os
import numpy as np

import concourse.bacc as bacc
import concourse.mybir as mybir
from concourse.tile import TileContext
from concourse.bass_utils import run_bass_kernel_spmd
from concourse.mybir import AluOpType as Op

B, L = 131072, 400
NCORES = 8
RPC = B // NCORES      # rows (neurons) per core
P = 128                # SBUF partitions
J = RPC // P           # neurons per partition = 128 (one step = [P, J] slice)

# Chunk schedule: small first chunks to fill the pipe fast, small last to
# drain fast. Sums to L.
CHUNKS = [8, 16, 32] + [64] * 5 + [16, 8]
assert sum(CHUNKS) == L

G = int(os.environ.get("BASS_LIF_G", "2"))        # interleaved groups
PLANES = os.environ.get("BASS_LIF_PLANES", "sig8")
JG = J // G

DECAY_MUL95 = 0.95
DECAY_MUL05 = 0.05
TH = 1.0
EPS = 1e-4            # sign2 band half-width
KSIG = 8192.0         # sig8 sigmoid sharpness

_nc_cache = None
_lif_op = None


def _register_lif_op():
    """Register the fused LIF-step custom DVE op (idempotent):
    out = in0 + (in1 * (in1 < s1)) * s0
    """
    global _lif_op
    if _lif_op is not None:
        return _lif_op
    import concourse.dve_ops as dve_ops
    from concourse.dve_spec import Spec, Src0, Src1, C0, C1, lower
    from concourse.dve_spec import _has_src1
    from concourse.dve_uop import DveOpSpec

    name = "LIF_STEP_ANT"
    for op in dve_ops.OPS:
        if op.name == name:
            _lif_op = op
            return op
    body = Src0 + (Src1 * (Src1 < C1)) * C0
    spec = Spec(
        body=body,
        reference=lambda in0, in1, s0, s1, imm2: (
            in0 + (in1 * (in1 < s1).astype(np.float32)) * np.float32(s0)
        ).astype(np.float32),
    )
    sha = {}
    for ver in ("v3", "v4"):
        sha[ver] = DveOpSpec(
            name=name, opcode=0x1F, uops=lower(spec, ver=ver),
            rd1_en=_has_src1(spec),
        ).sha(ver)
    op = dve_ops.DveOp(name, spec, subdim=False, uops_sha=sha)
    dve_ops.OPS.append(op)
    dve_ops.CUSTOM_DVE_SPECS[name] = spec
    row = dve_ops._CUSTOM_DVE_ROW_BASE + len(dve_ops.OPS) - 1
    assert row < 0x20, "custom-DVE opcode rows exhausted"
    dve_ops._SUB_OPCODE_FOR_NAME[name] = row
    _lif_op = op
    return op


def _build():
    nc = bacc.Bacc(None, target_bir_lowering=False)
    X = nc.dram_tensor("X", [P, L * J], mybir.dt.float32, kind="ExternalInput")
    if PLANES == "sig8":
        S8 = nc.dram_tensor("S8", [P, L * J], mybir.dt.float8e4, kind="ExternalOutput")
    else:
        A = nc.dram_tensor("A", [P, L * J], mybir.dt.uint8, kind="ExternalOutput")
        Bp = nc.dram_tensor("Bq", [P, L * J], mybir.dt.uint8, kind="ExternalOutput")
    lif = _register_lif_op()

    with TileContext(nc) as tc:
        with (
            tc.tile_pool(name="state", bufs=1) as state_pool,
            tc.tile_pool(name="io", bufs=4) as io_pool,
            tc.tile_pool(name="pl", bufs=2) as pl_pool,
        ):
            if PLANES == "sig8":
                bsg = state_pool.tile([P, 1], mybir.dt.float32, name="bsg")
                nc.vector.memset(bsg[:], -KSIG)
            else:
                blo = state_pool.tile([P, 1], mybir.dt.float32, name="blo")
                bhi = state_pool.tile([P, 1], mybir.dt.float32, name="bhi")
                nc.vector.memset(blo[:], float(EPS - TH))
                nc.vector.memset(bhi[:], float(-EPS - TH))
            prev_tile, prev_t = None, 0
            t0 = 0
            for ch, TC in enumerate(CHUNKS):
                base = t0 * J
                t0 += TC
                xin = io_pool.tile([P, TC * J], mybir.dt.float32, name="xin")
                nc.sync.dma_start(xin[:], X[:, base : base + TC * J])
                for t in range(TC):
                    if prev_tile is None:
                        # v init is 0, so u_0 = I_0: the DMA'd input slice
                        # already is u_0 -- skip the first step's op.
                        prev_tile, prev_t = xin, 0
                        continue
                    sls = [
                        slice(t * J + g * JG, t * J + (g + 1) * JG) for g in range(G)
                    ]
                    psls = [
                        slice(prev_t * J + g * JG, prev_t * J + (g + 1) * JG)
                        for g in range(G)
                    ]
                    for g in range(G):
                        nc.vector._custom_dve(
                            lif,
                            out=xin[:, sls[g]],
                            in0=xin[:, sls[g]],
                            in1=prev_tile[:, psls[g]],
                            s0=DECAY_MUL95,
                            s1=TH,
                        )
                    prev_tile, prev_t = xin, t
                if PLANES == "sig8":
                    # sig = sigmoid(KSIG*(u-1)) -> fp8: monotone byte code of u;
                    # bytes near 0.5 (|u-1| <~ 4e-5) are the host-repair band.
                    ps = pl_pool.tile([P, TC * J], mybir.dt.float8e4, name="ps")
                    nc.scalar.activation(
                        ps[:], xin[:], mybir.ActivationFunctionType.Sigmoid,
                        bias=bsg[:], scale=KSIG,
                    )
                    nc.scalar.dma_start(S8[:, base : base + TC * J], ps[:])
                else:
                    pa = pl_pool.tile([P, TC * J], mybir.dt.uint8, name="pa")
                    pb = pl_pool.tile([P, TC * J], mybir.dt.uint8, name="pb")
                    nc.scalar.activation(
                        pa[:], xin[:], mybir.ActivationFunctionType.Sign,
                        bias=blo[:], scale=1.0,
                    )
                    nc.scalar.activation(
                        pb[:], xin[:], mybir.ActivationFunctionType.Sign,
                        bias=bhi[:], scale=1.0,
                    )
                    nc.scalar.dma_start(A[:, base : base + TC * J], pa[:])
                    nc.scalar.dma_start(Bp[:, base : base + TC * J], pb[:])
    nc.compile()
    return nc


def _get_nc():
    global _nc_cache
    if _nc_cache is None:
        _nc_cache = _build()
    return _nc_cache


def _shard(I):
    # Per-core host transposes run in parallel (numpy releases the GIL
    # during the strided copies).
    from concurrent.futures import ThreadPoolExecutor

    def one(c):
        Ic = I[c * RPC : (c + 1) * RPC]                    # [RPC, L]
        Xc = Ic.reshape(P, J, L).transpose(0, 2, 1)        # [P, L, J] time-major
        return {"X": np.ascontiguousarray(Xc).reshape(P, L * J)}

    with ThreadPoolExecutor(NCORES) as ex:
        return list(ex.map(one, range(NCORES)))


def _unshard_plane(results, key):
    from concurrent.futures import ThreadPoolExecutor

    out = np.empty((B, L), np.uint8)

    def one(c):
        r = np.asarray(results[c][key])
        if r.dtype != np.uint8:
            r = r.view(np.uint8) if r.dtype.itemsize == 1 else r.astype(np.uint8)
        Sc = r.reshape(P, L, J).transpose(0, 2, 1)         # [P, J, L]
        out[c * RPC : (c + 1) * RPC] = Sc.reshape(RPC, L)

    with ThreadPoolExecutor(NCORES) as ex:
        list(ex.map(one, range(NCORES)))
    return out


def _f8e4m3_lut():
    """byte -> float32 value of fp8 e4m3 (bias 7)."""
    b = np.arange(256, dtype=np.uint32)
    sign = np.where(b >> 7, -1.0, 1.0)
    e = (b >> 3) & 0xF
    m = b & 0x7
    val = np.where(
        e == 0,
        (m / 8.0) * 2.0 ** (-6),
        (1.0 + m / 8.0) * (2.0 ** (e.astype(np.int32) - 7)),
    )
    return (sign * val).astype(np.float32)


def _decode(I, results):
    f32 = np.float32
    if PLANES == "sig8":
        raw = _unshard_plane(results, "S8")
        val = _f8e4m3_lut()[raw]
        spikes = val > f32(0.5)
        suspect = np.abs(val - f32(0.5)) <= f32(0.05)
    else:
        pa = _unshard_plane(results, "A")
        pb = _unshard_plane(results, "Bq")
        spikes = pa == 1
        suspect = spikes & (pb != 1)
    rows = np.nonzero(suspect.any(axis=1))[0]
    out = spikes.astype(f32)
    if rows.size:
        # Bit-exact reference recurrence for the suspect rows only.
        Ir = I[rows]
        v = np.zeros(rows.size, f32)
        for t in range(L):
            u = ((v - v * f32(DECAY_MUL05)) + Ir[:, t]).astype(f32)
            s = u >= f32(TH)
            out[rows, t] = s
            v = np.where(s, f32(0.0), u)
    return out


def kernel(I, _trace=False):
    I = np.ascontiguousarray(np.asarray(I), dtype=np.float32)
    assert I.shape == (B, L), I.shape
    nc = _get_nc()
    br = run_bass_kernel_spmd(nc, _shard(I), core_ids=list(range(NCORES)), trace=_trace)
    out = _decode(I, br.results)
    if _trace:
        return out, br
    return out


# revision 13
# speedup vs baseline: 2.1741x; 1.0380x over previous
"""LIF neuron scan kernel for Trainium2, sharded over 8 NeuronCores.

Device recurrence, ONE custom DVE instruction per time step (f32):
    u_t = I_t + 0.95 * (u_{t-1} * (u_{t-1} < 1))
computed in-place over the input tile (state = previous u slice), via a
registered custom DVE op (4 ALU stages).  The mask-multiply is exact, so
this matches the fused form u = round(round(0.95*v)+I).

The fused decay differs from the reference's (v - v/20) by <= ~4e-6
over the whole trajectory (measured with synced resets), so the spike
raster can only flip where u lands within that distance of threshold.
The device emits one fp8-e4m3 plane sig = sigmoid(8192*(u-1)) (ACT
engine): bytes decode monotonically in u, sigma > 0.5 <=> u > 1, and any
u within 3.8e-6 of threshold maps within 0.008 of sigma=0.5 -- far
inside one fp8 quantum (0.0625) -- so the host flags bytes near 0.5 as
suspects and recomputes those rows bit-exactly with the reference
formula (~1e-5 of rows).

Sharding: batch dim B=131072 split into 8 contiguous blocks of 16384
rows. Per core the block is laid out time-major as [128 partitions, 400
steps, 128 neurons] so each step is one [128,128] SBUF slice and DMA
chunks are per-partition contiguous.
"""

import os
import numpy as np

import concourse.bacc as bacc
import concourse.mybir as mybir
from concourse.tile import TileContext
from concourse.bass_utils import run_bass_kernel_spmd
from concourse.mybir import AluOpType as Op

B, L = 131072, 400
NCORES = 8
RPC = B // NCORES      # rows (neurons) per core
P = 128                # SBUF partitions
J = RPC // P           # neurons per partition = 128 (one step = [P, J] slice)

# Chunk schedule: small first chunks to fill the pipe fast, small last to
# drain fast. Sums to L.
CHUNKS = [8, 16, 32] + [64] * 5 + [16, 8]
assert sum(CHUNKS) == L

G = int(os.environ.get("BASS_LIF_G", "2"))        # interleaved groups
PLANES = os.environ.get("BASS_LIF_PLANES", "sig8")
JG = J // G

DECAY_MUL95 = 0.95
DECAY_MUL05 = 0.05
TH = 1.0
EPS = 1e-4            # sign2 band half-width
KSIG = 8192.0         # sig8 sigmoid sharpness

_nc_cache = None
_lif_op = None


def _register_lif_op():
    """Register the fused LIF-step custom DVE op (idempotent):
    out = in0 + (in1 * (in1 < s1)) * s0
    """
    global _lif_op
    if _lif_op is not None:
        return _lif_op
    import concourse.dve_ops as dve_ops
    from concourse.dve_spec import Spec, Src0, Src1, C0, C1, lower
    from concourse.dve_spec import _has_src1
    from concourse.dve_uop import DveOpSpec

    name = "LIF_STEP_ANT"
    for op in dve_ops.OPS:
        if op.name == name:
            _lif_op = op
            return op
    body = Src0 + (Src1 * (Src1 < C1)) * C0
    spec = Spec(
        body=body,
        reference=lambda in0, in1, s0, s1, imm2: (
            in0 + (in1 * (in1 < s1).astype(np.float32)) * np.float32(s0)
        ).astype(np.float32),
    )
    sha = {}
    for ver in ("v3", "v4"):
        sha[ver] = DveOpSpec(
            name=name, opcode=0x1F, uops=lower(spec, ver=ver),
            rd1_en=_has_src1(spec),
        ).sha(ver)
    op = dve_ops.DveOp(name, spec, subdim=False, uops_sha=sha)
    dve_ops.OPS.append(op)
    dve_ops.CUSTOM_DVE_SPECS[name] = spec
    row = dve_ops._CUSTOM_DVE_ROW_BASE + len(dve_ops.OPS) - 1
    assert row < 0x20, "custom-DVE opcode rows exhausted"
    dve_ops._SUB_OPCODE_FOR_NAME[name] = row
    _lif_op = op
    return op


def _build():
    nc = bacc.Bacc(None, target_bir_lowering=False)
    X = nc.dram_tensor("X", [P, L * J], mybir.dt.float32, kind="ExternalInput")
    if PLANES == "sig8":
        S8 = nc.dram_tensor("S8", [P, L * J], mybir.dt.float8e4, kind="ExternalOutput")
    else:
        A = nc.dram_tensor("A", [P, L * J], mybir.dt.uint8, kind="ExternalOutput")
        Bp = nc.dram_tensor("Bq", [P, L * J], mybir.dt.uint8, kind="ExternalOutput")
    lif = _register_lif_op()

    with TileContext(nc) as tc:
        with (
            tc.tile_pool(name="state", bufs=1) as state_pool,
            tc.tile_pool(name="io", bufs=5) as io_pool,
            tc.tile_pool(name="pl", bufs=3) as pl_pool,
        ):
            if PLANES == "sig8":
                bsg = state_pool.tile([P, 1], mybir.dt.float32, name="bsg")
                nc.vector.memset(bsg[:], -KSIG)
            else:
                blo = state_pool.tile([P, 1], mybir.dt.float32, name="blo")
                bhi = state_pool.tile([P, 1], mybir.dt.float32, name="bhi")
                nc.vector.memset(blo[:], float(EPS - TH))
                nc.vector.memset(bhi[:], float(-EPS - TH))
            prev_tile, prev_t = None, 0
            t0 = 0
            for ch, TC in enumerate(CHUNKS):
                base = t0 * J
                t0 += TC
                xin = io_pool.tile([P, TC * J], mybir.dt.float32, name="xin")
                nc.sync.dma_start(xin[:], X[:, base : base + TC * J])
                for t in range(TC):
                    if prev_tile is None:
                        # v init is 0, so u_0 = I_0: the DMA'd input slice
                        # already is u_0 -- skip the first step's op.
                        prev_tile, prev_t = xin, 0
                        continue
                    sls = [
                        slice(t * J + g * JG, t * J + (g + 1) * JG) for g in range(G)
                    ]
                    psls = [
                        slice(prev_t * J + g * JG, prev_t * J + (g + 1) * JG)
                        for g in range(G)
                    ]
                    for g in range(G):
                        nc.vector._custom_dve(
                            lif,
                            out=xin[:, sls[g]],
                            in0=xin[:, sls[g]],
                            in1=prev_tile[:, psls[g]],
                            s0=DECAY_MUL95,
                            s1=TH,
                        )
                    prev_tile, prev_t = xin, t
                if PLANES == "sig8":
                    # sig = sigmoid(KSIG*(u-1)) -> fp8: monotone byte code of u;
                    # bytes near 0.5 (|u-1| <~ 4e-5) are the host-repair band.
                    ps = pl_pool.tile([P, TC * J], mybir.dt.float8e4, name="ps")
                    nc.scalar.activation(
                        ps[:], xin[:], mybir.ActivationFunctionType.Sigmoid,
                        bias=bsg[:], scale=KSIG,
                    )
                    nc.scalar.dma_start(S8[:, base : base + TC * J], ps[:])
                else:
                    pa = pl_pool.tile([P, TC * J], mybir.dt.uint8, name="pa")
                    pb = pl_pool.tile([P, TC * J], mybir.dt.uint8, name="pb")
                    nc.scalar.activation(
                        pa[:], xin[:], mybir.ActivationFunctionType.Sign,
                        bias=blo[:], scale=1.0,
                    )
                    nc.scalar.activation(
                        pb[:], xin[:], mybir.ActivationFunctionType.Sign,
                        bias=bhi[:], scale=1.0,
                    )
                    nc.scalar.dma_start(A[:, base : base + TC * J], pa[:])
                    nc.scalar.dma_start(Bp[:, base : base + TC * J], pb[:])
    nc.compile()
    return nc


def _get_nc():
    global _nc_cache
    if _nc_cache is None:
        _nc_cache = _build()
    return _nc_cache


def _shard(I):
    # Per-core host transposes run in parallel (numpy releases the GIL
    # during the strided copies).
    from concurrent.futures import ThreadPoolExecutor

    def one(c):
        Ic = I[c * RPC : (c + 1) * RPC]                    # [RPC, L]
        Xc = Ic.reshape(P, J, L).transpose(0, 2, 1)        # [P, L, J] time-major
        return {"X": np.ascontiguousarray(Xc).reshape(P, L * J)}

    with ThreadPoolExecutor(NCORES) as ex:
        return list(ex.map(one, range(NCORES)))


def _unshard_plane(results, key):
    from concurrent.futures import ThreadPoolExecutor

    out = np.empty((B, L), np.uint8)

    def one(c):
        r = np.asarray(results[c][key])
        if r.dtype != np.uint8:
            r = r.view(np.uint8) if r.dtype.itemsize == 1 else r.astype(np.uint8)
        Sc = r.reshape(P, L, J).transpose(0, 2, 1)         # [P, J, L]
        out[c * RPC : (c + 1) * RPC] = Sc.reshape(RPC, L)

    with ThreadPoolExecutor(NCORES) as ex:
        list(ex.map(one, range(NCORES)))
    return out


def _f8e4m3_lut():
    """byte -> float32 value of fp8 e4m3 (bias 7)."""
    b = np.arange(256, dtype=np.uint32)
    sign = np.where(b >> 7, -1.0, 1.0)
    e = (b >> 3) & 0xF
    m = b & 0x7
    val = np.where(
        e == 0,
        (m / 8.0) * 2.0 ** (-6),
        (1.0 + m / 8.0) * (2.0 ** (e.astype(np.int32) - 7)),
    )
    return (sign * val).astype(np.float32)


def _decode(I, results):
    f32 = np.float32
    if PLANES == "sig8":
        raw = _unshard_plane(results, "S8")
        val = _f8e4m3_lut()[raw]
        spikes = val > f32(0.5)
        suspect = np.abs(val - f32(0.5)) <= f32(0.05)
    else:
        pa = _unshard_plane(results, "A")
        pb = _unshard_plane(results, "Bq")
        spikes = pa == 1
        suspect = spikes & (pb != 1)
    rows = np.nonzero(suspect.any(axis=1))[0]
    out = spikes.astype(f32)
    if rows.size:
        # Bit-exact reference recurrence for the suspect rows only.
        Ir = I[rows]
        v = np.zeros(rows.size, f32)
        for t in range(L):
            u = ((v - v * f32(DECAY_MUL05)) + Ir[:, t]).astype(f32)
            s = u >= f32(TH)
            out[rows, t] = s
            v = np.where(s, f32(0.0), u)
    return out


def kernel(I, _trace=False):
    I = np.ascontiguousarray(np.asarray(I), dtype=np.float32)
    assert I.shape == (B, L), I.shape
    nc = _get_nc()
    br = run_bass_kernel_spmd(nc, _shard(I), core_ids=list(range(NCORES)), trace=_trace)
    out = _decode(I, br.results)
    if _trace:
        return out, br
    return out
